# revision 1
# baseline (speedup 1.0000x reference)
"""Trainium2 Bass kernel for HardQuadRadiusTripletLoss.

Computes: per-keypoint dense correlation (2048x256 @ 256x3600 per image),
geometric radius masking (cells whose center is within 8px of the warped
keypoint), top-4 hard negatives, positive-cell similarity, and the
squared-hinge triplet loss reduced to a scalar.

Sharding: data-parallel over batch B=8 -> one image per NeuronCore.

Device pipeline per 128-keypoint tile (16 tiles/core), per 450-col chunk (8):
  PE  : d2m64 = [dy^2|dx^2|1]^T @ bpat      (f32r matmul -> dist2 - 64 in PSUM)
  ACT : u = relu(-K*(d2m64))                (K=2^20; f32r out; 0 outside mask)
  PE  : sim  = kp1_descT.T @ desc2          (f32r, 2 k-passes, PSUM)
        sim += (-I) @ u                     (neg-identity matmul applies mask)
  DVE : chunk top-8 = vector.max(sim_psum)  -> merge buffer
Per tile: DVE merge max over 8x8 chunk maxima -> top-8; indirect row-gather of
desc2T[flat_idx] + fused dot (scalar_tensor_tensor accum) -> positive sim.
Host: input transposes / coordinate prep, final relu(neg-pos+1)^2 mean.
"""

import sys

if "/opt/trn_rl_repo" not in sys.path:
    sys.path.insert(0, "/opt/trn_rl_repo")

import numpy as np

B, N, C, H, W = 8, 2048, 256, 60, 60
HW = H * W            # 3600
GRID = 8.0
NTILE = N // 128      # 16
NCHUNK = 8
CH = HW // NCHUNK     # 450
KPEN = float(2 ** 20)

_NC_CACHE = {}


def _build_nc():
    from concourse import bacc, mybir, bass
    import concourse.tile as tile

    nc = bacc.Bacc("TRN2", target_bir_lowering=False, debug=False)
    f32 = mybir.dt.float32
    f32r = mybir.dt.float32r
    i32 = mybir.dt.int32
    Alu = mybir.AluOpType
    Act = mybir.ActivationFunctionType

    d_desc2f = nc.dram_tensor("desc2f", (C, HW), f32, kind="ExternalInput").ap()
    d_desc2T = nc.dram_tensor("desc2T", (HW, C), f32, kind="ExternalInput").ap()
    d_kpT = nc.dram_tensor("kpT", (C, N), f32, kind="ExternalInput").ap()
    d_kpn = nc.dram_tensor("kpn", (N, C), f32, kind="ExternalInput").ap()
    d_dyxT = nc.dram_tensor("dyxT", (121, N), f32, kind="ExternalInput").ap()
    d_bpat = nc.dram_tensor("bpat", (121, HW), f32, kind="ExternalInput").ap()
    d_negid = nc.dram_tensor("negid", (128, 128), f32, kind="ExternalInput").ap()
    d_fidx = nc.dram_tensor("fidx", (N, 1), i32, kind="ExternalInput").ap()
    d_top8 = nc.dram_tensor("top8", (N, 8), f32, kind="ExternalOutput").ap()
    d_pos = nc.dram_tensor("pos", (N, 1), f32, kind="ExternalOutput").ap()

    with tile.TileContext(nc) as tc:
        with (
            tc.tile_pool(name="pers", bufs=1) as pers,
            tc.tile_pool(name="stage", bufs=2) as stage,
            tc.tile_pool(name="upool", bufs=3) as upool,
            tc.tile_pool(name="work", bufs=3) as work,
            tc.tile_pool(name="ps_d", bufs=2, space="PSUM") as ps_d,
            tc.tile_pool(name="ps_s", bufs=4, space="PSUM") as ps_s,
        ):
            # Persistent f32r operands: DMA load then the mandatory
            # f32r-rounding cast (DVE).
            def load_r(nm, dram_ap, shape):
                st = stage.tile(list(shape), f32, tag="stage")
                nc.sync.dma_start(st[:], dram_ap)
                tr = pers.tile(list(shape), f32r, tag=nm)
                nc.vector.tensor_copy(tr[:], st[:])
                return tr

            dyxT = load_r("dyxT", d_dyxT[:], (121, N))
            bp = load_r("bp", d_bpat[:], (121, HW))
            kpT0 = load_r("kpT0", d_kpT[0:128, :], (128, N))
            kpT1 = load_r("kpT1", d_kpT[128:256, :], (128, N))
            rhs0 = load_r("rhs0", d_desc2f[0:128, :], (128, HW))
            rhs1 = load_r("rhs1", d_desc2f[128:256, :], (128, HW))
            negid = load_r("negid", d_negid[:], (128, 128))

            for t in range(NTILE):
                ns = slice(t * 128, (t + 1) * 128)

                # ---- positive similarity path (exact fp32) ----
                kpn_t = work.tile([128, C], f32, tag="kpn")
                nc.sync.dma_start(kpn_t[:], d_kpn[ns, :])
                fidx_t = work.tile([128, 1], i32, tag="fidx")
                nc.sync.dma_start(fidx_t[:], d_fidx[ns, :])
                posd_t = work.tile([128, C], f32, tag="posd")
                nc.gpsimd.indirect_dma_start(
                    out=posd_t[:],
                    out_offset=None,
                    in_=d_desc2T[:],
                    in_offset=bass.IndirectOffsetOnAxis(ap=fidx_t[:, :1], axis=0),
                )
                junk_t = work.tile([128, C], f32, tag="junk")
                pos_t = work.tile([128, 1], f32, tag="pos")
                nc.vector.scalar_tensor_tensor(
                    out=junk_t[:],
                    in0=posd_t[:],
                    scalar=1.0,
                    in1=kpn_t[:],
                    op0=Alu.mult,
                    op1=Alu.mult,
                    accum_out=pos_t[:],
                )
                nc.sync.dma_start(d_pos[ns, :], pos_t[:])

                # ---- dense correlation + mask + chunkwise top8 ----
                m64 = work.tile([128, 64], f32, tag="m64")
                for c in range(NCHUNK):
                    cs = slice(c * CH, (c + 1) * CH)
                    d2 = ps_d.tile([128, CH], f32, tag="d2")
                    nc.tensor.matmul(
                        out=d2[:], lhsT=dyxT[:, ns], rhs=bp[:, cs],
                        start=True, stop=True,
                    )
                    u = upool.tile([128, CH], f32r, tag="u")
                    nc.scalar.activation(
                        out=u[:], in_=d2[:], func=Act.Relu, scale=-KPEN,
                    )
                    sm = ps_s.tile([128, CH], f32, tag="sm")
                    nc.tensor.matmul(
                        out=sm[:], lhsT=kpT0[:, ns], rhs=rhs0[:, cs],
                        start=True, stop=False,
                    )
                    nc.tensor.matmul(
                        out=sm[:], lhsT=kpT1[:, ns], rhs=rhs1[:, cs],
                        start=False, stop=False,
                    )
                    nc.tensor.matmul(
                        out=sm[:], lhsT=negid[:], rhs=u[:],
                        start=False, stop=True,
                    )
                    nc.vector.max(out=m64[:, c * 8:(c + 1) * 8], in_=sm[:])

                top8_t = work.tile([128, 8], f32, tag="top8")
                nc.vector.max(out=top8_t[:], in_=m64[:])
                nc.sync.dma_start(d_top8[ns, :], top8_t[:])

    nc.compile()
    return nc


def get_nc():
    if "nc" not in _NC_CACHE:
        _NC_CACHE["nc"] = _build_nc()
    return _NC_CACHE["nc"]


def make_in_maps(w_kp1, kp1_desc, desc2):
    yc = ((np.arange(H, dtype=np.float32) + np.float32(0.5)) * np.float32(GRID))
    bpat = np.zeros((121, HW), np.float32)
    for h in range(H):
        bpat[h, h * W:(h + 1) * W] = 1.0
    for w in range(W):
        bpat[60 + w, w::W] = 1.0
    bpat[120, :] = -64.0
    negid = -np.eye(128, dtype=np.float32)

    in_maps = []
    for b in range(B):
        wb = np.asarray(w_kp1[b], dtype=np.float32)
        cy = np.clip(np.floor(wb[:, 0] / np.float32(GRID)).astype(np.int32), 0, H - 1)
        cx = np.clip(np.floor(wb[:, 1] / np.float32(GRID)).astype(np.int32), 0, W - 1)
        fidx = (cy * W + cx).astype(np.int32).reshape(N, 1)
        dy = wb[:, 0:1] - yc[None, :]
        dx = wb[:, 1:2] - yc[None, :]
        dyxT = np.empty((121, N), np.float32)
        dyxT[0:60] = (dy * dy).T
        dyxT[60:120] = (dx * dx).T
        dyxT[120] = 1.0
        kpd = np.ascontiguousarray(np.asarray(kp1_desc[b], dtype=np.float32))
        d2f = np.ascontiguousarray(np.asarray(desc2[b], dtype=np.float32).reshape(C, HW))
        in_maps.append({
            "desc2f": d2f,
            "desc2T": np.ascontiguousarray(d2f.T),
            "kpT": np.ascontiguousarray(kpd.T),
            "kpn": kpd,
            "dyxT": np.ascontiguousarray(dyxT),
            "bpat": bpat,
            "negid": negid,
            "fidx": fidx,
        })
    return in_maps


def finish_loss(results):
    total = 0.0
    for b in range(B):
        out = results[b]
        neg4 = out["top8"][:, :4].astype(np.float64)
        pos = out["pos"].astype(np.float64)
        t = np.maximum(neg4 - pos + 1.0, 0.0)
        total += float((t * t).sum())
    return np.asarray(np.float32(total / (B * N * 4)))


def kernel(kp1, w_kp1, kp1_desc, desc2, homo12):
    from concourse.bass_utils import run_bass_kernel_spmd

    nc = get_nc()
    in_maps = make_in_maps(w_kp1, kp1_desc, desc2)
    res = run_bass_kernel_spmd(nc, in_maps, core_ids=list(range(B)))
    return finish_loss(res.results)



# revision 8
# speedup vs baseline: 2.1133x; 2.1133x over previous
"""Trainium2 Bass kernel for HardQuadRadiusTripletLoss.

Per image (one per NeuronCore, B=8): dense correlation sim = kp1_desc
(2048x256) @ desc2 (256x3600) in fp16 on the PE, then a cross-chunk
elementwise-max fold tree + top-8 (DVE max8) per 128-keypoint tile.

Device pipeline per tile (16 tiles/core), chunks of 450 cols (8/tile):
  PE  : sim = kpT0.T@rhs0 + kpT1.T@rhs1   (fp16 in, f32 PSUM, 2 k-passes)
  ACT : groups 0-2 (6 chunks) PSUM f32 -> SBUF fp16 copies
  DVE : group 3 folded straight from PSUM; fp16 fold tree (2x mode) down
        to one 450-wide tile; max8 -> per-tile top-8 candidates
Radius masking + positive sim + final loss run on the host: the host
enumerates the <=4 masked cells per keypoint (grid-radius geometry),
recomputes their sims from the same fp16-rounded inputs, removes them
from the device top-8 by value match, and takes the top-4 negatives.
Keypoints left with <4 candidates fall back to an exact host recompute.
"""

import sys

if "/opt/trn_rl_repo" not in sys.path:
    sys.path.insert(0, "/opt/trn_rl_repo")

import numpy as np

B, N, C, H, W = 8, 2048, 256, 60, 60
HW = H * W            # 3600
GRID = 8.0
NTILE = N // 128      # 16
NCHUNK = 8
CH = HW // NCHUNK     # 450
PRE = 4               # keypoint tiles preloaded before the bulk kpT DMA
TOL = 2.5e-4          # |host fp32 sim - device fp16 sim| match tolerance

_NC_CACHE = {}


def _build_nc():
    from concourse import bacc, mybir
    import concourse.tile as tile

    nc = bacc.Bacc("TRN2", target_bir_lowering=False, debug=False)
    f32 = mybir.dt.float32
    f16 = mybir.dt.float16

    d_desc2h = nc.dram_tensor("desc2h", (C, HW), f16, kind="ExternalInput").ap()
    d_kpTh = nc.dram_tensor("kpTh", (C, N), f16, kind="ExternalInput").ap()
    d_top8 = nc.dram_tensor("top8", (128, NTILE * 16), f16, kind="ExternalOutput").ap()

    with tile.TileContext(nc) as tc:
        with (
            tc.tile_pool(name="pers", bufs=1) as pers,
            tc.tile_pool(name="gbuf", bufs=2) as gbuf,
            tc.tile_pool(name="fbuf", bufs=2) as fbuf,
            tc.tile_pool(name="ps", bufs=1, space="PSUM") as ps,
        ):
            rhs0 = pers.tile([128, HW], f16, tag="rhs0")
            rhs1 = pers.tile([128, HW], f16, tag="rhs1")
            kpT0 = pers.tile([128, N], f16, tag="kpT0")
            kpT1 = pers.tile([128, N], f16, tag="kpT1")
            outb = pers.tile([128, NTILE * 16], f16, tag="outb")

            # Prologue DMAs, ordered so tile 0 can start ASAP: first the
            # kpT slices for the first PRE tiles, then rhs chunk by chunk
            # (tile 0 consumes chunks in order), then the kpT remainder.
            nc.sync.dma_start(kpT0[:, 0 : PRE * 128], d_kpTh[0:128, 0 : PRE * 128])
            nc.sync.dma_start(kpT1[:, 0 : PRE * 128], d_kpTh[128:256, 0 : PRE * 128])
            for c in range(NCHUNK):
                cs = slice(c * CH, (c + 1) * CH)
                nc.sync.dma_start(rhs0[:, cs], d_desc2h[0:128, cs])
                nc.sync.dma_start(rhs1[:, cs], d_desc2h[128:256, cs])
            nc.sync.dma_start(kpT0[:, PRE * 128 :], d_kpTh[0:128, PRE * 128 :])
            nc.sync.dma_start(kpT1[:, PRE * 128 :], d_kpTh[128:256, PRE * 128 :])

            for t in range(NTILE):
                ns = slice(t * 128, (t + 1) * 128)

                # 4 groups x 2 chunks; [128, 2, 512] f32 = exactly 2 PSUM
                # banks per group, 8 banks total.
                psg = [
                    ps.tile([128, 2, 512], f32, tag=f"ps{g}", name=f"ps{g}")
                    for g in range(4)
                ]
                for g in range(4):
                    for cc in range(2):
                        c = g * 2 + cc
                        cs = slice(c * CH, (c + 1) * CH)
                        o = psg[g][:, cc, 0:CH]
                        nc.tensor.matmul(
                            out=o, lhsT=kpT0[:, ns], rhs=rhs0[:, cs],
                            start=True, stop=False,
                        )
                        nc.tensor.matmul(
                            out=o, lhsT=kpT1[:, ns], rhs=rhs1[:, cs],
                            start=False, stop=True,
                        )

                # ACT: convert groups 0-2 (chunks 0-5) to one fp16 tile.
                G = gbuf.tile([128, 6, CH], f16, tag="G")
                for g in range(3):
                    nc.scalar.copy(G[:, 2 * g : 2 * g + 2, :], psg[g][:, :, 0:CH])

                # DVE: top-8 of chunks 6-7 straight from PSUM (exact), and a
                # fold tree over the 6 ACT chunks (fp16 2x mode) + top-8.
                nc.vector.max(
                    outb[:, t * 16 + 8 : t * 16 + 16], psg[3][:, :, 0:CH]
                )
                Wf = fbuf.tile([128, 3, CH], f16, tag="Wf")
                nc.vector.tensor_max(Wf[:], G[:, 0:3, :], G[:, 3:6, :])
                V = fbuf.tile([128, CH], f16, tag="V")
                nc.vector.tensor_max(V[:], Wf[:, 0, :], Wf[:, 1, :])
                V2 = fbuf.tile([128, CH], f16, tag="V2")
                nc.vector.tensor_max(V2[:], V[:], Wf[:, 2, :])
                nc.vector.max(outb[:, t * 16 : t * 16 + 8], V2[:])

            nc.sync.dma_start(d_top8[:], outb[:])

    nc.compile()
    return nc


def get_nc():
    if "nc" not in _NC_CACHE:
        _NC_CACHE["nc"] = _build_nc()
    return _NC_CACHE["nc"]


def make_in_maps(w_kp1, kp1_desc, desc2):
    in_maps = []
    for b in range(B):
        kpd = np.asarray(kp1_desc[b], dtype=np.float32)
        d2f = np.asarray(desc2[b], dtype=np.float32).reshape(C, HW)
        in_maps.append({
            "desc2h": np.ascontiguousarray(d2f.astype(np.float16)),
            "kpTh": np.ascontiguousarray(kpd.T.astype(np.float16)),
        })
    return in_maps


def _host_image_loss(top8_dev, w, kpd, d2f):
    """Sum of squared-hinge terms for one image (not yet averaged)."""
    # Device candidate layout: [128 partitions, 16 tiles * 16]; keypoint
    # t*128+p lives at [p, t*16:(t+1)*16]: first 8 = fold-tree top-8 of
    # chunks 0-5, next 8 = exact top-8 of chunks 6-7.
    cand = (
        top8_dev.reshape(128, NTILE, 16)
        .transpose(1, 0, 2)
        .reshape(N, 16)
        .astype(np.float64)
    )

    # fp16-rounded copies: match device matmul inputs bit-for-bit.
    kph = kpd.astype(np.float16).astype(np.float32)
    d2h = d2f.astype(np.float16).astype(np.float32)

    # --- positive sim (exact fp32, like the reference) ---
    cy = np.clip(np.floor(w[:, 0] / np.float32(GRID)).astype(np.int64), 0, H - 1)
    cx = np.clip(np.floor(w[:, 1] / np.float32(GRID)).astype(np.int64), 0, W - 1)
    fidx = cy * W + cx
    pos = np.einsum("nc,cn->n", kpd, d2f[:, fidx]).astype(np.float64)

    # --- masked cells: centers within GRID px of the warped keypoint ---
    yc = (np.arange(H, dtype=np.float32) + np.float32(0.5)) * np.float32(GRID)
    offs = np.array([-2, -1, 0, 1], np.int64)
    hb = np.floor(w[:, 0] / np.float32(GRID)).astype(np.int64)[:, None] + offs
    wb = np.floor(w[:, 1] / np.float32(GRID)).astype(np.int64)[:, None] + offs
    vh = (hb >= 0) & (hb < H)
    vw = (wb >= 0) & (wb < W)
    hcc = np.clip(hb, 0, H - 1)
    wcc = np.clip(wb, 0, W - 1)
    dy = w[:, 0:1] - yc[hcc]
    dx = w[:, 1:2] - yc[wcc]
    d2 = dy[:, :, None] ** 2 + dx[:, None, :] ** 2
    m = (d2 <= np.float32(GRID * GRID)) & vh[:, :, None] & vw[:, None, :]
    nn, ii, jj = np.nonzero(m)
    cells = hcc[nn, ii] * W + wcc[nn, jj]
    mvals = np.einsum("kc,ck->k", kph[nn], d2h[:, cells]).astype(np.float64)

    # Bucket masked values per keypoint (nn is ascending from nonzero).
    first = np.searchsorted(nn, np.arange(N))
    posin = np.arange(len(nn)) - first[nn]
    mv_mat = np.full((N, 16), -np.inf)
    mv_mat[nn, posin] = mvals

    # --- remove masked values from the candidates ---
    avail = np.ones((N, cand.shape[1]), bool)
    ar = np.arange(N)
    for s in range(mv_mat.shape[1]):
        mv = mv_mat[:, s]
        has = np.isfinite(mv)
        if not has.any():
            break
        diff = np.abs(cand - mv[:, None])
        diff[~avail] = np.inf
        j = np.argmin(diff, axis=1)
        hit = has & (diff[ar, j] <= TOL)
        avail[hit, j[hit]] = False

    vals = np.where(avail, cand, -np.inf)
    vals = -np.sort(-vals, axis=1)
    neg4 = vals[:, :4]

    # --- fallback: exact recompute for keypoints left with <4 candidates ---
    deficient = np.nonzero(~np.isfinite(neg4).any(axis=1) | ~np.isfinite(neg4[:, 3]))[0]
    for n in deficient:
        simr = (kph[n][None, :] @ d2h).ravel().astype(np.float64)
        dyf = w[n, 0] - yc
        dxf = w[n, 1] - yc
        d2full = (dyf[:, None] ** 2 + dxf[None, :] ** 2).reshape(-1)
        simr[d2full <= np.float32(GRID * GRID)] = -1e4
        neg4[n] = np.sort(simr)[-4:][::-1]

    t = np.maximum(neg4 - pos[:, None] + 1.0, 0.0)
    return float((t * t).sum())


def finish_loss(results, w_kp1, kp1_desc, desc2):
    total = 0.0
    for b in range(B):
        total += _host_image_loss(
            np.asarray(results[b]["top8"]),
            np.asarray(w_kp1[b], dtype=np.float32),
            np.asarray(kp1_desc[b], dtype=np.float32),
            np.asarray(desc2[b], dtype=np.float32).reshape(C, HW),
        )
    return np.asarray(np.float32(total / (B * N * 4)))


def kernel(kp1, w_kp1, kp1_desc, desc2, homo12):
    from concourse.bass_utils import run_bass_kernel_spmd

    nc = get_nc()
    in_maps = make_in_maps(w_kp1, kp1_desc, desc2)
    res = run_bass_kernel_spmd(nc, in_maps, core_ids=list(range(B)))
    return finish_loss(res.results, w_kp1, kp1_desc, desc2)


# revision 12
# speedup vs baseline: 2.1462x; 1.0155x over previous
"""Trainium2 Bass kernel for HardQuadRadiusTripletLoss.

Per image (one per NeuronCore, B=8): dense correlation sim = kp1_desc
(2048x256) @ desc2 (256x3600) in fp16 on the PE, then a cross-chunk
elementwise-max fold tree + top-8 (DVE max8) per 128-keypoint tile.

Device pipeline per tile (16 tiles/core), chunks of 450 cols (8/tile):
  PE  : sim = kpT0.T@rhs0 + kpT1.T@rhs1   (fp16 in, f32 PSUM, 2 k-passes)
  ACT : groups 0-2 (6 chunks) PSUM f32 -> SBUF fp16 copies
  DVE : group 3 folded straight from PSUM; fp16 fold tree (2x mode) down
        to one 450-wide tile; max8 -> per-tile top-8 candidates
Radius masking + positive sim + final loss run on the host: the host
enumerates the <=4 masked cells per keypoint (grid-radius geometry),
recomputes their sims from the same fp16-rounded inputs, removes them
from the device top-8 by value match, and takes the top-4 negatives.
Keypoints left with <4 candidates fall back to an exact host recompute.
"""

import sys

if "/opt/trn_rl_repo" not in sys.path:
    sys.path.insert(0, "/opt/trn_rl_repo")

import numpy as np

B, N, C, H, W = 8, 2048, 256, 60, 60
HW = H * W            # 3600
GRID = 8.0
NTILE = N // 128      # 16
NCHUNK = 8
CH = HW // NCHUNK     # 450
PRE = 4               # keypoint tiles preloaded before the bulk kpT DMA
TOL = 2.5e-4          # |host fp32 sim - device fp16 sim| match tolerance

_NC_CACHE = {}


def _build_nc():
    from concourse import bacc, mybir
    import concourse.tile as tile

    nc = bacc.Bacc("TRN2", target_bir_lowering=False, debug=False)
    f32 = mybir.dt.float32
    f16 = mybir.dt.float16

    d_desc2h = nc.dram_tensor("desc2h", (C, HW), f16, kind="ExternalInput").ap()
    d_kpTh = nc.dram_tensor("kpTh", (C, N), f16, kind="ExternalInput").ap()
    # tiles 0..14: 16 cols each (tree top8 + direct top8); tile 15: 32 cols
    # (4 direct top8s, no ACT dependency -> short tail).
    d_top8 = nc.dram_tensor(
        "top8", (128, 15 * 16 + 32), f16, kind="ExternalOutput"
    ).ap()

    with tile.TileContext(nc) as tc:
        with (
            tc.tile_pool(name="pers", bufs=1) as pers,
            tc.tile_pool(name="gbuf", bufs=2) as gbuf,
            tc.tile_pool(name="fbuf", bufs=2) as fbuf,
            tc.tile_pool(name="ps", bufs=1, space="PSUM") as ps,
        ):
            rhs0 = pers.tile([128, HW], f16, tag="rhs0")
            rhs1 = pers.tile([128, HW], f16, tag="rhs1")
            kpT0 = pers.tile([128, N], f16, tag="kpT0")
            kpT1 = pers.tile([128, N], f16, tag="kpT1")
            outb = pers.tile([128, 15 * 16 + 32], f16, tag="outb")

            # Prologue on two DMA queues (SP + ACT) so the per-instruction
            # overheads overlap: kpT slices for the first tiles, then rhs in
            # halves (tile 0 consumes chunks in order), then the kpT rest.
            HWH = HW // 2
            nc.sync.dma_start(kpT0[:, 0 : PRE * 128], d_kpTh[0:128, 0 : PRE * 128])
            nc.sync.dma_start(rhs0[:, 0:HWH], d_desc2h[0:128, 0:HWH])
            nc.sync.dma_start(rhs0[:, HWH:HW], d_desc2h[0:128, HWH:HW])
            nc.sync.dma_start(kpT0[:, PRE * 128 :], d_kpTh[0:128, PRE * 128 :])
            nc.scalar.dma_start(kpT1[:, 0 : PRE * 128], d_kpTh[128:256, 0 : PRE * 128])
            nc.scalar.dma_start(rhs1[:, 0:HWH], d_desc2h[128:256, 0:HWH])
            nc.scalar.dma_start(rhs1[:, HWH:HW], d_desc2h[128:256, HWH:HW])
            nc.scalar.dma_start(kpT1[:, PRE * 128 :], d_kpTh[128:256, PRE * 128 :])

            for t in range(NTILE):
                ns = slice(t * 128, (t + 1) * 128)

                # 4 groups x 2 chunks; [128, 2, 512] f32 = exactly 2 PSUM
                # banks per group, 8 banks total.
                psg = [
                    ps.tile([128, 2, 512], f32, tag=f"ps{g}", name=f"ps{g}")
                    for g in range(4)
                ]
                for g in range(4):
                    for cc in range(2):
                        c = g * 2 + cc
                        cs = slice(c * CH, (c + 1) * CH)
                        o = psg[g][:, cc, 0:CH]
                        nc.tensor.matmul(
                            out=o, lhsT=kpT0[:, ns], rhs=rhs0[:, cs],
                            start=True, stop=False,
                        )
                        nc.tensor.matmul(
                            out=o, lhsT=kpT1[:, ns], rhs=rhs1[:, cs],
                            start=False, stop=True,
                        )

                if t == NTILE - 1:
                    # Last tile: all four groups via direct PSUM max8 so the
                    # epilogue has no ACT->fold chain in it.
                    for g in range(4):
                        nc.vector.max(
                            outb[:, 240 + 8 * g : 240 + 8 * g + 8],
                            psg[g][:, :, 0:CH],
                        )
                    continue

                # ACT: convert groups 0-2 (chunks 0-5) to one fp16 tile.
                G = gbuf.tile([128, 6, CH], f16, tag="G")
                for g in range(3):
                    nc.scalar.copy(G[:, 2 * g : 2 * g + 2, :], psg[g][:, :, 0:CH])

                # DVE: top-8 of chunks 6-7 straight from PSUM (exact), and a
                # fold tree over the 6 ACT chunks (fp16 2x mode) + top-8.
                nc.vector.max(
                    outb[:, t * 16 + 8 : t * 16 + 16], psg[3][:, :, 0:CH]
                )
                Wf = fbuf.tile([128, 3, CH], f16, tag="Wf")
                nc.vector.tensor_max(Wf[:], G[:, 0:3, :], G[:, 3:6, :])
                V = fbuf.tile([128, CH], f16, tag="V")
                nc.vector.tensor_max(V[:], Wf[:, 0, :], Wf[:, 1, :])
                V2 = fbuf.tile([128, CH], f16, tag="V2")
                nc.vector.tensor_max(V2[:], V[:], Wf[:, 2, :])
                nc.vector.max(outb[:, t * 16 : t * 16 + 8], V2[:])
                if t == NTILE - 2:
                    # Ship tiles 0..14 while the last tile still computes.
                    nc.sync.dma_start(d_top8[:, 0:240], outb[:, 0:240])

            nc.sync.dma_start(d_top8[:, 240:272], outb[:, 240:272])

    nc.compile()
    return nc


def get_nc():
    if "nc" not in _NC_CACHE:
        _NC_CACHE["nc"] = _build_nc()
    return _NC_CACHE["nc"]


def make_in_maps(w_kp1, kp1_desc, desc2):
    in_maps = []
    for b in range(B):
        kpd = np.asarray(kp1_desc[b], dtype=np.float32)
        d2f = np.asarray(desc2[b], dtype=np.float32).reshape(C, HW)
        in_maps.append({
            "desc2h": np.ascontiguousarray(d2f.astype(np.float16)),
            "kpTh": np.ascontiguousarray(kpd.T.astype(np.float16)),
        })
    return in_maps


def _host_image_loss(top8_dev, w, kpd, d2f):
    """Sum of squared-hinge terms for one image (not yet averaged)."""
    # Device candidate layout: [128 partitions, 272]. Tiles 0..14 have 16
    # cols at [t*16, t*16+16) (fold-tree top-8 of chunks 0-5, then exact
    # top-8 of chunks 6-7); tile 15 has 32 cols at [240, 272) (four exact
    # per-group top-8s). Pad to a uniform [N, 32] with -inf.
    cand = np.full((NTILE, 128, 32), -np.inf)
    body = top8_dev[:, 0:240].astype(np.float64).reshape(128, 15, 16)
    cand[0:15, :, 0:16] = body.transpose(1, 0, 2)
    cand[15, :, :] = top8_dev[:, 240:272].astype(np.float64)
    cand = cand.reshape(N, 32)

    # fp16-rounded copies: match device matmul inputs bit-for-bit.
    kph = kpd.astype(np.float16).astype(np.float32)
    d2h = d2f.astype(np.float16).astype(np.float32)

    # --- positive sim (exact fp32, like the reference) ---
    cy = np.clip(np.floor(w[:, 0] / np.float32(GRID)).astype(np.int64), 0, H - 1)
    cx = np.clip(np.floor(w[:, 1] / np.float32(GRID)).astype(np.int64), 0, W - 1)
    fidx = cy * W + cx
    pos = np.einsum("nc,cn->n", kpd, d2f[:, fidx]).astype(np.float64)

    # --- masked cells: centers within GRID px of the warped keypoint ---
    yc = (np.arange(H, dtype=np.float32) + np.float32(0.5)) * np.float32(GRID)
    offs = np.array([-2, -1, 0, 1], np.int64)
    hb = np.floor(w[:, 0] / np.float32(GRID)).astype(np.int64)[:, None] + offs
    wb = np.floor(w[:, 1] / np.float32(GRID)).astype(np.int64)[:, None] + offs
    vh = (hb >= 0) & (hb < H)
    vw = (wb >= 0) & (wb < W)
    hcc = np.clip(hb, 0, H - 1)
    wcc = np.clip(wb, 0, W - 1)
    dy = w[:, 0:1] - yc[hcc]
    dx = w[:, 1:2] - yc[wcc]
    d2 = dy[:, :, None] ** 2 + dx[:, None, :] ** 2
    m = (d2 <= np.float32(GRID * GRID)) & vh[:, :, None] & vw[:, None, :]
    nn, ii, jj = np.nonzero(m)
    cells = hcc[nn, ii] * W + wcc[nn, jj]
    mvals = np.einsum("kc,ck->k", kph[nn], d2h[:, cells]).astype(np.float64)

    # Bucket masked values per keypoint (nn is ascending from nonzero).
    first = np.searchsorted(nn, np.arange(N))
    posin = np.arange(len(nn)) - first[nn]
    mv_mat = np.full((N, 16), -np.inf)
    mv_mat[nn, posin] = mvals

    # --- remove masked values from the candidates ---
    avail = np.ones((N, cand.shape[1]), bool)
    ar = np.arange(N)
    for s in range(mv_mat.shape[1]):
        mv = mv_mat[:, s]
        has = np.isfinite(mv)
        if not has.any():
            break
        diff = np.abs(cand - mv[:, None])
        diff[~avail] = np.inf
        j = np.argmin(diff, axis=1)
        hit = has & (diff[ar, j] <= TOL)
        avail[hit, j[hit]] = False

    vals = np.where(avail, cand, -np.inf)
    vals = -np.sort(-vals, axis=1)
    neg4 = vals[:, :4]

    # --- fallback: exact recompute for keypoints left with <4 candidates ---
    deficient = np.nonzero(~np.isfinite(neg4).any(axis=1) | ~np.isfinite(neg4[:, 3]))[0]
    for n in deficient:
        simr = (kph[n][None, :] @ d2h).ravel().astype(np.float64)
        dyf = w[n, 0] - yc
        dxf = w[n, 1] - yc
        d2full = (dyf[:, None] ** 2 + dxf[None, :] ** 2).reshape(-1)
        simr[d2full <= np.float32(GRID * GRID)] = -1e4
        neg4[n] = np.sort(simr)[-4:][::-1]

    t = np.maximum(neg4 - pos[:, None] + 1.0, 0.0)
    return float((t * t).sum())


def finish_loss(results, w_kp1, kp1_desc, desc2):
    total = 0.0
    for b in range(B):
        total += _host_image_loss(
            np.asarray(results[b]["top8"]),
            np.asarray(w_kp1[b], dtype=np.float32),
            np.asarray(kp1_desc[b], dtype=np.float32),
            np.asarray(desc2[b], dtype=np.float32).reshape(C, HW),
        )
    return np.asarray(np.float32(total / (B * N * 4)))


def kernel(kp1, w_kp1, kp1_desc, desc2, homo12):
    from concourse.bass_utils import run_bass_kernel_spmd

    nc = get_nc()
    in_maps = make_in_maps(w_kp1, kp1_desc, desc2)
    res = run_bass_kernel_spmd(nc, in_maps, core_ids=list(range(B)))
    return finish_loss(res.results, w_kp1, kp1_desc, desc2)


# revision 17
# speedup vs baseline: 2.1764x; 1.0141x over previous
"""Trainium2 Bass kernel for HardQuadRadiusTripletLoss.

Per image (one per NeuronCore, B=8): dense correlation sim = kp1_desc
(2048x256) @ desc2 (256x3600) in fp16 on the PE, then a cross-chunk
elementwise-max fold tree + top-8 (DVE max8) per 128-keypoint tile.

Device pipeline per tile (16 tiles/core), chunks of 450 cols (8/tile):
  PE  : sim = kpT0.T@rhs0 + kpT1.T@rhs1   (fp16 in, f32 PSUM, 2 k-passes)
  ACT : groups 0-2 (6 chunks) PSUM f32 -> SBUF fp16 copies
  DVE : group 3 folded straight from PSUM; fp16 fold tree (2x mode) down
        to one 450-wide tile; max8 -> per-tile top-8 candidates
Radius masking + positive sim + final loss run on the host: the host
enumerates the <=4 masked cells per keypoint (grid-radius geometry),
recomputes their sims from the same fp16-rounded inputs, removes them
from the device top-8 by value match, and takes the top-4 negatives.
Keypoints left with <4 candidates fall back to an exact host recompute.
"""

import sys

if "/opt/trn_rl_repo" not in sys.path:
    sys.path.insert(0, "/opt/trn_rl_repo")

import numpy as np

B, N, C, H, W = 8, 2048, 256, 60, 60
HW = H * W            # 3600
GRID = 8.0
NTILE = N // 128      # 16
NCHUNK = 8
CH = HW // NCHUNK     # 450
PRE = 4               # keypoint tiles preloaded before the bulk kpT DMA
TOL = 2.5e-4          # |host fp32 sim - device fp16 sim| match tolerance

_NC_CACHE = {}


def _build_nc():
    from concourse import bacc, mybir
    import concourse.tile as tile

    nc = bacc.Bacc("TRN2", target_bir_lowering=False, debug=False)
    f32 = mybir.dt.float32
    f16 = mybir.dt.float16

    d_desc2h = nc.dram_tensor("desc2h", (C, HW), f16, kind="ExternalInput").ap()
    d_kpTh = nc.dram_tensor("kpTh", (C, N), f16, kind="ExternalInput").ap()
    # tiles 0..14: 16 cols each (tree top8 + direct top8); tile 15: 8 cols
    # (one 8-chunk fold-tree top8, ACT-converted -> short tail).
    d_top8 = nc.dram_tensor(
        "top8", (128, 15 * 16 + 8), f16, kind="ExternalOutput"
    ).ap()

    with tile.TileContext(nc) as tc:
        with (
            tc.tile_pool(name="pers", bufs=1) as pers,
            tc.tile_pool(name="gbuf", bufs=2) as gbuf,
            tc.tile_pool(name="fbuf", bufs=2) as fbuf,
            tc.tile_pool(name="ps", bufs=1, space="PSUM") as ps,
        ):
            rhs0 = pers.tile([128, HW], f16, tag="rhs0")
            rhs1 = pers.tile([128, HW], f16, tag="rhs1")
            kpT0 = pers.tile([128, N], f16, tag="kpT0")
            kpT1 = pers.tile([128, N], f16, tag="kpT1")
            outb = pers.tile([128, 15 * 16 + 8], f16, tag="outb")

            # Prologue on two DMA queues (SP + ACT) so the per-instruction
            # overheads overlap. rhs goes in chunk-pair slices so tile 0 can
            # chase the loads group by group.
            nc.sync.dma_start(kpT0[:, 0 : PRE * 128], d_kpTh[0:128, 0 : PRE * 128])
            nc.scalar.dma_start(
                kpT1[:, 0 : PRE * 128], d_kpTh[128:256, 0 : PRE * 128]
            )
            for g in range(4):
                gs = slice(g * 2 * CH, (g + 1) * 2 * CH)
                nc.sync.dma_start(rhs0[:, gs], d_desc2h[0:128, gs])
                nc.scalar.dma_start(rhs1[:, gs], d_desc2h[128:256, gs])
            nc.sync.dma_start(kpT0[:, PRE * 128 :], d_kpTh[0:128, PRE * 128 :])
            nc.scalar.dma_start(kpT1[:, PRE * 128 :], d_kpTh[128:256, PRE * 128 :])

            for t in range(NTILE):
                ns = slice(t * 128, (t + 1) * 128)

                # 4 groups x 2 chunks; [128, 2, 512] f32 = exactly 2 PSUM
                # banks per group, 8 banks total.
                psg = [
                    ps.tile([128, 2, 512], f32, tag=f"ps{g}", name=f"ps{g}")
                    for g in range(4)
                ]
                for g in range(4):
                    for cc in range(2):
                        c = g * 2 + cc
                        cs = slice(c * CH, (c + 1) * CH)
                        o = psg[g][:, cc, 0:CH]
                        nc.tensor.matmul(
                            out=o, lhsT=kpT0[:, ns], rhs=rhs0[:, cs],
                            start=True, stop=False,
                        )
                        nc.tensor.matmul(
                            out=o, lhsT=kpT1[:, ns], rhs=rhs1[:, cs],
                            start=False, stop=True,
                        )

                if t == NTILE - 1:
                    # Last tile: ACT (idle by now) converts all four groups;
                    # DVE runs one 8-chunk fold tree. Keeps the epilogue off
                    # the loaded DVE stream.
                    G8 = gbuf.tile([128, 8, CH], f16, tag="G8")
                    for g in range(4):
                        nc.scalar.copy(
                            G8[:, 2 * g : 2 * g + 2, :], psg[g][:, :, 0:CH]
                        )
                    W8 = fbuf.tile([128, 4, CH], f16, tag="W8")
                    nc.vector.tensor_max(W8[:], G8[:, 0:4, :], G8[:, 4:8, :])
                    X8 = fbuf.tile([128, 2, CH], f16, tag="X8")
                    nc.vector.tensor_max(X8[:], W8[:, 0:2, :], W8[:, 2:4, :])
                    Y8 = fbuf.tile([128, CH], f16, tag="Y8")
                    nc.vector.tensor_max(Y8[:], X8[:, 0, :], X8[:, 1, :])
                    nc.vector.max(outb[:, 240:248], Y8[:])
                    continue

                # ACT: convert groups 0-2 (chunks 0-5) to one fp16 tile.
                G = gbuf.tile([128, 6, CH], f16, tag="G")
                for g in range(3):
                    nc.scalar.copy(G[:, 2 * g : 2 * g + 2, :], psg[g][:, :, 0:CH])

                # DVE: top-8 of chunks 6-7 straight from PSUM (exact), and a
                # fold tree over the 6 ACT chunks (fp16 2x mode) + top-8.
                nc.vector.max(
                    outb[:, t * 16 + 8 : t * 16 + 16], psg[3][:, :, 0:CH]
                )
                Wf = fbuf.tile([128, 3, CH], f16, tag="Wf")
                nc.vector.tensor_max(Wf[:], G[:, 0:3, :], G[:, 3:6, :])
                V = fbuf.tile([128, CH], f16, tag="V")
                nc.vector.tensor_max(V[:], Wf[:, 0, :], Wf[:, 1, :])
                V2 = fbuf.tile([128, CH], f16, tag="V2")
                nc.vector.tensor_max(V2[:], V[:], Wf[:, 2, :])
                nc.vector.max(outb[:, t * 16 : t * 16 + 8], V2[:])
                if t == NTILE - 2:
                    # Ship tiles 0..14 while the last tile still computes.
                    nc.sync.dma_start(d_top8[:, 0:240], outb[:, 0:240])

            nc.sync.dma_start(d_top8[:, 240:248], outb[:, 240:248])

    nc.compile()
    return nc


def get_nc():
    if "nc" not in _NC_CACHE:
        _NC_CACHE["nc"] = _build_nc()
    return _NC_CACHE["nc"]


def make_in_maps(w_kp1, kp1_desc, desc2):
    in_maps = []
    for b in range(B):
        kpd = np.asarray(kp1_desc[b], dtype=np.float32)
        d2f = np.asarray(desc2[b], dtype=np.float32).reshape(C, HW)
        in_maps.append({
            "desc2h": np.ascontiguousarray(d2f.astype(np.float16)),
            "kpTh": np.ascontiguousarray(kpd.T.astype(np.float16)),
        })
    return in_maps


def _host_image_loss(top8_dev, w, kpd, d2f):
    """Sum of squared-hinge terms for one image (not yet averaged)."""
    # Device candidate layout: [128 partitions, 248]. Tiles 0..14 have 16
    # cols at [t*16, t*16+16) (fold-tree top-8 of chunks 0-5, then exact
    # top-8 of chunks 6-7); tile 15 has 8 cols at [240, 248) (8-chunk
    # fold-tree top-8). Pad to a uniform [N, 16] with -inf.
    cand = np.full((NTILE, 128, 16), -np.inf)
    body = top8_dev[:, 0:240].astype(np.float64).reshape(128, 15, 16)
    cand[0:15] = body.transpose(1, 0, 2)
    cand[15, :, 0:8] = top8_dev[:, 240:248].astype(np.float64)
    cand = cand.reshape(N, 16)

    # fp16-rounded copies: match device matmul inputs bit-for-bit.
    kph = kpd.astype(np.float16).astype(np.float32)
    d2h = d2f.astype(np.float16).astype(np.float32)

    # --- positive sim (exact fp32, like the reference) ---
    cy = np.clip(np.floor(w[:, 0] / np.float32(GRID)).astype(np.int64), 0, H - 1)
    cx = np.clip(np.floor(w[:, 1] / np.float32(GRID)).astype(np.int64), 0, W - 1)
    fidx = cy * W + cx
    pos = np.einsum("nc,cn->n", kpd, d2f[:, fidx]).astype(np.float64)

    # --- masked cells: centers within GRID px of the warped keypoint ---
    yc = (np.arange(H, dtype=np.float32) + np.float32(0.5)) * np.float32(GRID)
    offs = np.array([-2, -1, 0, 1], np.int64)
    hb = np.floor(w[:, 0] / np.float32(GRID)).astype(np.int64)[:, None] + offs
    wb = np.floor(w[:, 1] / np.float32(GRID)).astype(np.int64)[:, None] + offs
    vh = (hb >= 0) & (hb < H)
    vw = (wb >= 0) & (wb < W)
    hcc = np.clip(hb, 0, H - 1)
    wcc = np.clip(wb, 0, W - 1)
    dy = w[:, 0:1] - yc[hcc]
    dx = w[:, 1:2] - yc[wcc]
    d2 = dy[:, :, None] ** 2 + dx[:, None, :] ** 2
    m = (d2 <= np.float32(GRID * GRID)) & vh[:, :, None] & vw[:, None, :]
    nn, ii, jj = np.nonzero(m)
    cells = hcc[nn, ii] * W + wcc[nn, jj]
    mvals = np.einsum("kc,ck->k", kph[nn], d2h[:, cells]).astype(np.float64)

    # Bucket masked values per keypoint (nn is ascending from nonzero).
    first = np.searchsorted(nn, np.arange(N))
    posin = np.arange(len(nn)) - first[nn]
    mv_mat = np.full((N, 16), -np.inf)
    mv_mat[nn, posin] = mvals

    # --- remove masked values from the candidates ---
    avail = np.ones((N, cand.shape[1]), bool)
    ar = np.arange(N)
    for s in range(mv_mat.shape[1]):
        mv = mv_mat[:, s]
        has = np.isfinite(mv)
        if not has.any():
            break
        diff = np.abs(cand - mv[:, None])
        diff[~avail] = np.inf
        j = np.argmin(diff, axis=1)
        hit = has & (diff[ar, j] <= TOL)
        avail[hit, j[hit]] = False

    vals = np.where(avail, cand, -np.inf)
    vals = -np.sort(-vals, axis=1)
    neg4 = vals[:, :4]

    # --- fallback: exact recompute for keypoints left with <4 candidates ---
    deficient = np.nonzero(~np.isfinite(neg4).any(axis=1) | ~np.isfinite(neg4[:, 3]))[0]
    for n in deficient:
        simr = (kph[n][None, :] @ d2h).ravel().astype(np.float64)
        dyf = w[n, 0] - yc
        dxf = w[n, 1] - yc
        d2full = (dyf[:, None] ** 2 + dxf[None, :] ** 2).reshape(-1)
        simr[d2full <= np.float32(GRID * GRID)] = -1e4
        neg4[n] = np.sort(simr)[-4:][::-1]

    t = np.maximum(neg4 - pos[:, None] + 1.0, 0.0)
    return float((t * t).sum())


def finish_loss(results, w_kp1, kp1_desc, desc2):
    total = 0.0
    for b in range(B):
        total += _host_image_loss(
            np.asarray(results[b]["top8"]),
            np.asarray(w_kp1[b], dtype=np.float32),
            np.asarray(kp1_desc[b], dtype=np.float32),
            np.asarray(desc2[b], dtype=np.float32).reshape(C, HW),
        )
    return np.asarray(np.float32(total / (B * N * 4)))


def kernel(kp1, w_kp1, kp1_desc, desc2, homo12):
    from concourse.bass_utils import run_bass_kernel_spmd

    nc = get_nc()
    in_maps = make_in_maps(w_kp1, kp1_desc, desc2)
    res = run_bass_kernel_spmd(nc, in_maps, core_ids=list(range(B)))
    return finish_loss(res.results, w_kp1, kp1_desc, desc2)


# revision 18
# speedup vs baseline: 2.2928x; 1.0535x over previous
"""Trainium2 Bass kernel for HardQuadRadiusTripletLoss.

Per image (one per NeuronCore, B=8): dense correlation sim = kp1_desc
(2048x256) @ desc2 (256x3600) on the PE in fp8e4m3 DoubleRow mode (the
K=256 contraction folds into one pass, 0.5 cycles/row), inputs
pre-scaled by 16 so e4m3 sees a well-conditioned range (sim lands in
PSUM scaled by 256). Readout per 128-keypoint tile:
  ACT : chunks 0-5 PSUM f32 -> SBUF fp16 with a fused 1/256 downscale
  DVE : top-8 of chunks 6-7 straight from PSUM (x256 scale, fixed on
        host); fp16 fold tree (2x mode) over the 6 ACT chunks + top-8
Radius masking + positive sim + final loss run on the host: the host
enumerates the <=4 masked cells per keypoint (grid-radius geometry),
recomputes their sims from the same fp8-quantized inputs, removes them
from the device candidates by value match, and takes the top-4
negatives. Keypoints left with <4 candidates fall back to an exact
host recompute. The positive similarity uses the original f32 inputs.
"""

import sys

if "/opt/trn_rl_repo" not in sys.path:
    sys.path.insert(0, "/opt/trn_rl_repo")

import numpy as np
import ml_dtypes

B, N, C, H, W = 8, 2048, 256, 60, 60
HW = H * W            # 3600
GRID = 8.0
NTILE = N // 128      # 16
NCHUNK = 8
CH = HW // NCHUNK     # 450
PRE = 4               # keypoint tiles preloaded before the bulk kpT DMA
SCALE = 16.0          # per-input fp8 pre-scale; sim is scaled by SCALE^2
TOL = 2.5e-4          # |host sim - device fp16 sim| match tolerance
F8 = ml_dtypes.float8_e4m3

_NC_CACHE = {}


def _build_nc():
    from concourse import bacc, mybir
    import concourse.tile as tile

    nc = bacc.Bacc("TRN2", target_bir_lowering=False, debug=False)
    f32 = mybir.dt.float32
    f16 = mybir.dt.float16
    f8 = mybir.dt.float8e4

    d_desc2q = nc.dram_tensor("desc2q", (C, HW), f8, kind="ExternalInput").ap()
    d_kpTq = nc.dram_tensor("kpTq", (C, N), f8, kind="ExternalInput").ap()
    # tiles 0..14: 16 cols each (tree top8 + direct top8); tile 15: 8 cols
    # (one 8-chunk fold-tree top8, ACT-converted -> short tail).
    d_top8 = nc.dram_tensor(
        "top8", (128, 15 * 16 + 8), f16, kind="ExternalOutput"
    ).ap()

    ISCALE = 1.0 / (SCALE * SCALE)

    with tile.TileContext(nc) as tc:
        with (
            tc.tile_pool(name="pers", bufs=1) as pers,
            tc.tile_pool(name="gbuf", bufs=2) as gbuf,
            tc.tile_pool(name="fbuf", bufs=2) as fbuf,
            tc.tile_pool(name="ps", bufs=1, space="PSUM") as ps,
        ):
            rhs8 = pers.tile([128, 2, HW], f8, tag="rhs8")
            kpT8 = pers.tile([128, 2, N], f8, tag="kpT8")
            outb = pers.tile([128, 15 * 16 + 8], f16, tag="outb")

            # Prologue on two DMA queues (SP + ACT): kpT head slices, rhs
            # in halves per K-half, then the kpT rest.
            HWH = HW // 2
            nc.sync.dma_start(
                kpT8[:, 0, 0 : PRE * 128], d_kpTq[0:128, 0 : PRE * 128]
            )
            nc.scalar.dma_start(
                kpT8[:, 1, 0 : PRE * 128], d_kpTq[128:256, 0 : PRE * 128]
            )
            nc.sync.dma_start(rhs8[:, 0, 0:HWH], d_desc2q[0:128, 0:HWH])
            nc.scalar.dma_start(rhs8[:, 1, 0:HWH], d_desc2q[128:256, 0:HWH])
            nc.sync.dma_start(rhs8[:, 0, HWH:HW], d_desc2q[0:128, HWH:HW])
            nc.scalar.dma_start(rhs8[:, 1, HWH:HW], d_desc2q[128:256, HWH:HW])
            nc.sync.dma_start(
                kpT8[:, 0, PRE * 128 :], d_kpTq[0:128, PRE * 128 :]
            )
            nc.scalar.dma_start(
                kpT8[:, 1, PRE * 128 :], d_kpTq[128:256, PRE * 128 :]
            )

            for t in range(NTILE):
                ns = slice(t * 128, (t + 1) * 128)

                # 4 groups x 2 chunks; [128, 2, 512] f32 = exactly 2 PSUM
                # banks per group, 8 banks total.
                psg = [
                    ps.tile([128, 2, 512], f32, tag=f"ps{g}", name=f"ps{g}")
                    for g in range(4)
                ]
                for g in range(4):
                    for cc in range(2):
                        c = g * 2 + cc
                        cs = slice(c * CH, (c + 1) * CH)
                        nc.tensor.matmul(
                            out=psg[g][:, cc, 0:CH],
                            lhsT=kpT8[:, :, ns],
                            rhs=rhs8[:, :, cs],
                            start=True, stop=True,
                            perf_mode=mybir.MatmulPerfMode.DoubleRow,
                        )

                if t == NTILE - 1:
                    # Last tile: ACT (idle by now) converts all four groups;
                    # DVE runs one 8-chunk fold tree. Keeps the epilogue off
                    # the loaded DVE stream.
                    G8 = gbuf.tile([128, 8, CH], f16, tag="G8")
                    for g in range(4):
                        nc.scalar.mul(
                            G8[:, 2 * g : 2 * g + 2, :], psg[g][:, :, 0:CH],
                            ISCALE,
                        )
                    W8 = fbuf.tile([128, 4, CH], f16, tag="W8")
                    nc.vector.tensor_max(W8[:], G8[:, 0:4, :], G8[:, 4:8, :])
                    X8 = fbuf.tile([128, 2, CH], f16, tag="X8")
                    nc.vector.tensor_max(X8[:], W8[:, 0:2, :], W8[:, 2:4, :])
                    Y8 = fbuf.tile([128, CH], f16, tag="Y8")
                    nc.vector.tensor_max(Y8[:], X8[:, 0, :], X8[:, 1, :])
                    nc.vector.max(outb[:, 240:248], Y8[:])
                    continue

                # ACT: convert chunks 0-5 to one fp16 tile, undoing the fp8
                # input pre-scale on the way.
                G = gbuf.tile([128, 6, CH], f16, tag="G")
                for g in range(3):
                    nc.scalar.mul(
                        G[:, 2 * g : 2 * g + 2, :], psg[g][:, :, 0:CH], ISCALE
                    )

                # DVE: top-8 of chunks 6-7 straight from PSUM (exact, still
                # x256-scaled), and a fold tree over the 6 ACT chunks.
                nc.vector.max(
                    outb[:, t * 16 + 8 : t * 16 + 16], psg[3][:, :, 0:CH]
                )
                Wf = fbuf.tile([128, 3, CH], f16, tag="Wf")
                nc.vector.tensor_max(Wf[:], G[:, 0:3, :], G[:, 3:6, :])
                V = fbuf.tile([128, CH], f16, tag="V")
                nc.vector.tensor_max(V[:], Wf[:, 0, :], Wf[:, 1, :])
                V2 = fbuf.tile([128, CH], f16, tag="V2")
                nc.vector.tensor_max(V2[:], V[:], Wf[:, 2, :])
                nc.vector.max(outb[:, t * 16 : t * 16 + 8], V2[:])
                if t == NTILE - 2:
                    # Ship tiles 0..14 while the last tile still computes.
                    nc.sync.dma_start(d_top8[:, 0:240], outb[:, 0:240])

            nc.sync.dma_start(d_top8[:, 240:248], outb[:, 240:248])

    nc.compile()
    return nc


def get_nc():
    if "nc" not in _NC_CACHE:
        _NC_CACHE["nc"] = _build_nc()
    return _NC_CACHE["nc"]


def make_in_maps(w_kp1, kp1_desc, desc2):
    in_maps = []
    for b in range(B):
        kpd = np.asarray(kp1_desc[b], dtype=np.float32)
        d2f = np.asarray(desc2[b], dtype=np.float32).reshape(C, HW)
        in_maps.append({
            "desc2q": np.ascontiguousarray((d2f * SCALE).astype(F8)),
            "kpTq": np.ascontiguousarray((kpd.T * SCALE).astype(F8)),
        })
    return in_maps


def _host_image_loss(top8_dev, w, kpd, d2f):
    """Sum of squared-hinge terms for one image (not yet averaged)."""
    # Device candidate layout: [128 partitions, 248]. Tiles 0..14 have 16
    # cols at [t*16, t*16+16): fold-tree top-8 (true scale), then direct
    # top-8 of chunks 6-7 (x256 scale). Tile 15 has 8 cols at [240, 248)
    # (8-chunk fold-tree top-8, true scale). Pad to [N, 16] with -inf.
    dev = top8_dev.astype(np.float64)
    cand = np.full((NTILE, 128, 16), -np.inf)
    body = dev[:, 0:240].reshape(128, 15, 16)
    cand[0:15] = body.transpose(1, 0, 2)
    cand[0:15, :, 8:16] /= SCALE * SCALE
    cand[15, :, 0:8] = dev[:, 240:248]
    cand = cand.reshape(N, 16)

    # fp8-quantized scaled copies: match device matmul inputs bit-for-bit.
    kph = (kpd * np.float32(SCALE)).astype(F8).astype(np.float32)
    d2h = (d2f * np.float32(SCALE)).astype(F8).astype(np.float32)

    # --- positive sim (exact fp32, like the reference) ---
    cy = np.clip(np.floor(w[:, 0] / np.float32(GRID)).astype(np.int64), 0, H - 1)
    cx = np.clip(np.floor(w[:, 1] / np.float32(GRID)).astype(np.int64), 0, W - 1)
    fidx = cy * W + cx
    pos = np.einsum("nc,cn->n", kpd, d2f[:, fidx]).astype(np.float64)

    # --- masked cells: centers within GRID px of the warped keypoint ---
    yc = (np.arange(H, dtype=np.float32) + np.float32(0.5)) * np.float32(GRID)
    offs = np.array([-2, -1, 0, 1], np.int64)
    hb = np.floor(w[:, 0] / np.float32(GRID)).astype(np.int64)[:, None] + offs
    wb = np.floor(w[:, 1] / np.float32(GRID)).astype(np.int64)[:, None] + offs
    vh = (hb >= 0) & (hb < H)
    vw = (wb >= 0) & (wb < W)
    hcc = np.clip(hb, 0, H - 1)
    wcc = np.clip(wb, 0, W - 1)
    dy = w[:, 0:1] - yc[hcc]
    dx = w[:, 1:2] - yc[wcc]
    d2 = dy[:, :, None] ** 2 + dx[:, None, :] ** 2
    m = (d2 <= np.float32(GRID * GRID)) & vh[:, :, None] & vw[:, None, :]
    nn, ii, jj = np.nonzero(m)
    cells = hcc[nn, ii] * W + wcc[nn, jj]
    mvals = np.einsum("kc,ck->k", kph[nn], d2h[:, cells]).astype(np.float64)
    mvals /= SCALE * SCALE

    # Bucket masked values per keypoint (nn is ascending from nonzero).
    first = np.searchsorted(nn, np.arange(N))
    posin = np.arange(len(nn)) - first[nn]
    mv_mat = np.full((N, 16), np.nan)
    mv_mat[nn, posin] = mvals

    # --- remove masked values from the candidates ---
    avail = np.ones((N, cand.shape[1]), bool)
    ar = np.arange(N)
    for s in range(mv_mat.shape[1]):
        mv = mv_mat[:, s]
        has = np.isfinite(mv)
        if not has.any():
            break
        diff = np.abs(np.where(avail, cand, np.inf) - np.where(has, mv, 0.0)[:, None])
        j = np.argmin(diff, axis=1)
        hit = has & (diff[ar, j] <= TOL)
        avail[hit, j[hit]] = False

    vals = np.where(avail, cand, -np.inf)
    vals = -np.sort(-vals, axis=1)
    neg4 = vals[:, :4]

    # --- fallback: exact recompute for keypoints left with <4 candidates ---
    deficient = np.nonzero(~np.isfinite(neg4[:, 3]))[0]
    for n in deficient:
        simr = (kph[n][None, :] @ d2h).ravel().astype(np.float64)
        simr /= SCALE * SCALE
        dyf = w[n, 0] - yc
        dxf = w[n, 1] - yc
        d2full = (dyf[:, None] ** 2 + dxf[None, :] ** 2).reshape(-1)
        simr[d2full <= np.float32(GRID * GRID)] = -1e4
        neg4[n] = np.sort(simr)[-4:][::-1]

    t = np.maximum(neg4 - pos[:, None] + 1.0, 0.0)
    return float((t * t).sum())


def finish_loss(results, w_kp1, kp1_desc, desc2):
    total = 0.0
    for b in range(B):
        total += _host_image_loss(
            np.asarray(results[b]["top8"]),
            np.asarray(w_kp1[b], dtype=np.float32),
            np.asarray(kp1_desc[b], dtype=np.float32),
            np.asarray(desc2[b], dtype=np.float32).reshape(C, HW),
        )
    return np.asarray(np.float32(total / (B * N * 4)))


def kernel(kp1, w_kp1, kp1_desc, desc2, homo12):
    from concourse.bass_utils import run_bass_kernel_spmd

    nc = get_nc()
    in_maps = make_in_maps(w_kp1, kp1_desc, desc2)
    res = run_bass_kernel_spmd(nc, in_maps, core_ids=list(range(B)))
    return finish_loss(res.results, w_kp1, kp1_desc, desc2)
